# revision 6
# baseline (speedup 1.0000x reference)
"""KLDivLoss(batchmean) of softmax(f1_rewards/tau) against log(output).

Contract: kernel(output=[1024,4096,1] f32, labels=[1024,4096] i32) -> () f32.

Math (per batch row):
    c_k  = cumsum(labels)            (k = 1..L)
    T    = c_L
    s_k  = (2/tau)*c_k / (k + T)     (== F1@k / tau; the where() guards in
                                      the reference collapse: c_k=0 => s_k=0.
                                      s in [0, ~1.18] -> exp safe without
                                      max-subtraction)
    q    = softmax(s);  Z = sum exp(s);  log q = s - ln Z
    row  = sum_k q_k*s_k - ln Z - sum_k q_k*ln p_k
    loss = sum_rows(row) / B

Distribution: pure data-parallel, 128 batch rows per NeuronCore (= the 128
SBUF partitions), 8 cores. Each core emits one f32 partial (its row-sum);
the host adds the 8 partials and divides by B.

Structure per core:
  - chunked local cumsum scans (independent, overlap the DMA); chunk carry
    offsets come from one tiny scan of the chunk totals and are folded into
    the s-computation's scalar slot (no extra pass)
  - kT = (iota + T)*(tau/2) one tensor_scalar; inv = reciprocal_approx_fast;
    s = (c_local + off)*inv via one scalar_tensor_tensor
  - s and ln(p) are written fp16, interleaved into one slab as
    [s_win(128) | lp_win(128)] per 128-wide window so ONE fp16 matmul per
    window against q computes both row-dot contractions (diagonal-block
    trick, PSUM fp32 accumulate)
  - row = (diag_a - lnZ) - diag_b; gpsimd partition reduce -> one scalar
"""

import numpy as np

B, L = 1024, 4096
N_CORES = 8
RPC = B // N_CORES  # rows per core = 128 = SBUF partitions
TAU = 0.85
CH = 1024  # free-dim chunk
NCH = L // CH
MM = 128  # matmul window
WPC = CH // MM  # windows per chunk
NWIN = L // MM

_NC_CACHE = {}


def build_nc():
    import concourse.bacc as bacc
    import concourse.bass_isa as bass_isa
    import concourse.mybir as mybir
    import concourse.tile as tile

    f32 = mybir.dt.float32
    f16 = mybir.dt.float16
    i32 = mybir.dt.int32
    Alu = mybir.AluOpType
    Act = mybir.ActivationFunctionType
    Ax = mybir.AxisListType

    nc = bacc.Bacc(
        "TRN2", target_bir_lowering=False, debug=False, num_devices=N_CORES
    )
    labels_d = nc.dram_tensor("labels", [RPC, L], i32, kind="ExternalInput").ap()
    p_d = nc.dram_tensor("p", [RPC, L], f32, kind="ExternalInput").ap()
    out_d = nc.dram_tensor("partial", [1, 1], f32, kind="ExternalOutput").ap()

    with tile.TileContext(nc) as tc:
        with (
            tc.tile_pool(name="persist", bufs=1) as persist,
            tc.tile_pool(name="lab", bufs=3) as lab_pool,
            tc.tile_pool(name="pin", bufs=3) as p_pool,
            tc.tile_pool(name="tmp", bufs=2) as tmp_pool,
            tc.tile_pool(name="small", bufs=1) as small,
            tc.tile_pool(name="psum", bufs=1, space="PSUM") as psum_pool,
        ):
            iota_t = persist.tile([RPC, L], i32)
            nc.gpsimd.iota(
                iota_t[:], pattern=[[1, L]], base=1, channel_multiplier=0
            )
            # identity matrix for extracting the diagonal of PSUM blocks
            ident = persist.tile([MM, MM], f32)
            nc.gpsimd.memset(ident[:], 1.0)
            nc.gpsimd.affine_select(
                ident[:],
                ident[:],
                pattern=[[-1, MM]],
                compare_op=Alu.is_equal,
                fill=0.0,
                base=0,
                channel_multiplier=1,
            )

            c_full = persist.tile([RPC, L], f32)
            e_full = persist.tile([RPC, L], f32)
            # slab holds [s_win | lp_win] pairs, fp16
            slab = persist.tile([RPC, 2 * L], f16)
            s_view = slab[:].rearrange("p (w x) -> p w x", x=2 * MM)[:, :, 0:MM]
            lp_view = slab[:].rearrange("p (w x) -> p w x", x=2 * MM)[
                :, :, MM : 2 * MM
            ]
            Zc = small.tile([RPC, NCH], f32)
            tot = small.tile([RPC, NCH], f32)

            # Phase 1: stream in; independent local cumsum per chunk; ln(p).
            for j in range(NCH):
                sl = slice(j * CH, (j + 1) * CH)
                wsl = slice(j * WPC, (j + 1) * WPC)
                lab = lab_pool.tile([RPC, CH], i32, tag="lab")
                nc.sync.dma_start(lab[:], labels_d[:, sl])
                nc.vector.tensor_tensor_scan(
                    c_full[:, sl], lab[:], lab[:], 0.0, Alu.add, Alu.bypass
                )
                nc.vector.tensor_copy(
                    tot[:, j : j + 1], c_full[:, (j + 1) * CH - 1 : (j + 1) * CH]
                )
                pt = p_pool.tile([RPC, CH], f32, tag="p")
                nc.sync.dma_start(pt[:], p_d[:, sl])
                nc.scalar.activation(
                    lp_view[:, wsl, :],
                    pt[:].rearrange("p (w x) -> p w x", x=MM),
                    Act.Ln,
                )

            # chunk offsets: tiny inclusive scan of the chunk totals
            offs = small.tile([RPC, NCH], f32)
            nc.vector.tensor_tensor_scan(
                offs[:], tot[:], tot[:], 0.0, Alu.add, Alu.bypass
            )
            T_ap = offs[:, NCH - 1 : NCH]

            # Phase 2: inv = 1/((k+T)*tau/2); s = (c_local+off)*inv; e; Z.
            for j in range(NCH):
                sl = slice(j * CH, (j + 1) * CH)
                wsl = slice(j * WPC, (j + 1) * WPC)
                kT = tmp_pool.tile([RPC, CH], f32, tag="kT")
                nc.vector.tensor_scalar(
                    kT[:], iota_t[:, sl], T_ap, TAU / 2.0, Alu.add, Alu.mult
                )
                inv = tmp_pool.tile([RPC, CH], f32, tag="inv")
                nc.vector.reciprocal_approx_fast(inv[:], kT[:])
                off = 0.0 if j == 0 else offs[:, j - 1 : j]
                nc.vector.scalar_tensor_tensor(
                    s_view[:, wsl, :],
                    c_full[:, sl].rearrange("p (w x) -> p w x", x=MM),
                    off,
                    inv[:].rearrange("p (w x) -> p w x", x=MM),
                    Alu.add,
                    Alu.mult,
                )
                nc.scalar.activation(
                    e_full[:, sl].rearrange("p (w x) -> p w x", x=MM),
                    s_view[:, wsl, :],
                    Act.Exp,
                    accum_out=Zc[:, j : j + 1],
                )

            Z = small.tile([RPC, 1], f32)
            nc.vector.tensor_reduce(Z[:], Zc[:], Ax.X, Alu.add)
            invZ = small.tile([RPC, 1], f32)
            nc.vector.reciprocal_approx_fast(invZ[:], Z[:])
            lnZ = small.tile([RPC, 1], f32)
            nc.scalar.activation(lnZ[:], Z[:], Act.Ln)

            # Phase 3: q = e*invZ (fp16); one matmul per 128-window against
            # the [s|lp] slab accumulates both contractions' diag blocks.
            psum_ab = psum_pool.tile([MM, 2 * MM], f32)
            for j in range(NCH):
                sl = slice(j * CH, (j + 1) * CH)
                ep = tmp_pool.tile([RPC, CH], f16, tag="ep")
                nc.vector.tensor_scalar(
                    ep[:], e_full[:, sl], invZ[:], None, Alu.mult
                )
                for w in range(WPC):
                    g = j * WPC + w
                    nc.tensor.matmul(
                        psum_ab[:],
                        ep[:, w * MM : (w + 1) * MM],
                        slab[:, g * 2 * MM : (g + 1) * 2 * MM],
                        start=(g == 0),
                        stop=(g == NWIN - 1),
                    )

            scr_a = small.tile([MM, MM], f32)
            diag_a = small.tile([MM, 1], f32)
            nc.vector.scalar_tensor_tensor(
                scr_a[:], psum_ab[:, 0:MM], 1.0, ident[:], Alu.mult, Alu.mult,
                accum_out=diag_a[:],
            )
            scr_b = small.tile([MM, MM], f32)
            diag_b = small.tile([MM, 1], f32)
            nc.vector.scalar_tensor_tensor(
                scr_b[:], psum_ab[:, MM : 2 * MM], 1.0, ident[:], Alu.mult,
                Alu.mult, accum_out=diag_b[:],
            )

            u = small.tile([RPC, 1], f32)
            nc.vector.scalar_tensor_tensor(
                u[:], diag_a[:], lnZ[:], diag_b[:], Alu.subtract, Alu.subtract
            )
            res = small.tile([RPC, 1], f32)
            nc.gpsimd.partition_all_reduce(
                res[:], u[:], RPC, bass_isa.ReduceOp.add
            )
            nc.sync.dma_start(out_d[:, :], res[0:1, :])
    nc.compile()
    return nc


def get_nc():
    nc = _NC_CACHE.get("nc")
    if nc is None:
        nc = build_nc()
        _NC_CACHE["nc"] = nc
    return nc


def shard_inputs(output, labels):
    p = np.ascontiguousarray(
        np.asarray(output, dtype=np.float32).reshape(B, L)
    )
    lab = np.ascontiguousarray(np.asarray(labels, dtype=np.int32))
    return [
        {
            "labels": lab[i * RPC : (i + 1) * RPC],
            "p": p[i * RPC : (i + 1) * RPC],
        }
        for i in range(N_CORES)
    ]


def gather(results):
    total = np.float64(0.0)
    for r in results:
        total += np.float64(r["partial"].reshape(-1)[0])
    return np.array(total / B, dtype=np.float32)


def kernel(output, labels):
    from concourse.bass_utils import run_bass_kernel_spmd

    nc = get_nc()
    in_maps = shard_inputs(output, labels)
    res = run_bass_kernel_spmd(nc, in_maps, list(range(N_CORES)))
    return gather(res.results)
